# revision 21
# baseline (speedup 1.0000x reference)
"""Multi-head attention (B=2, T=2048, D=OUT=1024, H=16) on 8 TRN2 NeuronCores.

Sharding: data-parallel over batch (2 groups) x tensor-parallel over heads
(4 groups of 4 heads). Core c handles batch c//4, heads (c%4)*4..(c%4)*4+4.
Each core computes Q^T/K^T/V for its head group, streams softmax(QK^T)V
in transposed layout (keys on partitions), and a partial output projection
through its W_o row block. The host sums the 4 partials per batch and adds
b_o.

Device-side layout notes:
- x is fed transposed ([D, T]) so Q^T/K^T come straight out of the PE.
- The softmax denominator rides along as a 65th ones-column in V', so one
  matmul produces both attn_unnorm^T and the per-query denominator.
- Matmul operands are bf16 (fp32 PSUM accumulation): full PE clock and
  fast weight load; fp32r measured 2x slower.
- Every matmul contracts over K=128 (K=64 matmuls run at half clock).
  The K-side operand of each attention matmul is a per-head tile
  zero-padded to 128 rows.
- Emission is interleaved: one software-pipelined stream over 128
  attention units; filler work rides on non-transition units.

Perf notes (this revision):
- Junk warm-up matmuls (vector-memset source, no DMA dependency) keep the
  HAM clock gate open from ~4us so real matmuls run at 2.4 GHz.
- Host pre-arranges weights into the exact SBUF layout so all input DMAs
  are contiguous; x arrives in two column halves and the pre-stream is
  trimmed to Q0(cols 0:1024) + K0(keys 0:512) so the exp stream starts
  ~10us earlier.
- era2 builds K1/kth[2:4] before Q1 so the vector queue never blocks the
  h1->h2 transition; Q1's second query half moved into the qbp1 phase
  (which is exp-bound, not PE-bound).
- normalize runs broadcast+multiply on gpsimd (vector's FIFO previously
  serialized the PE's next-head dependencies behind it).
- Partial outputs are fp16 (halves the output traffic); host accumulates
  in fp32. Tail output projections alternate PSUM pools and staging
  engines and spread DMAs across queues.
"""

import numpy as np

import concourse.bass as bass
import concourse.mybir as mybir
import concourse.tile as tile
from concourse import bacc
from concourse.bass_utils import run_bass_kernel_spmd

B, T, D, OUT, H = 2, 2048, 1024, 1024, 16
DO = 256            # output columns per core (4 heads x 64)
DEPTH = 64
NH = 4              # heads per core
KT = D // 128       # 8 contraction tiles for the projections
TT = T // 128       # 16 key tiles
NB = T // 512       # 4 query/time blocks
F32 = mybir.dt.float32
F16 = mybir.dt.float16
BF16 = mybir.dt.bfloat16
MMDT = BF16
EXP = mybir.ActivationFunctionType.Exp
MULT = mybir.AluOpType.mult
ADD = mybir.AluOpType.add

N_WARM = 9          # junk matmuls to open the HAM clock gate

_CACHE = {}


def build_attention(nc, dbg=None):
    """Emit the attention program. dbg, when given, is a dict of extra DRAM
    debug outputs to dump intermediates into."""
    xt = nc.declare_dram_parameter("xt", [D, T], MMDT, isOutput=False)
    wq = nc.declare_dram_parameter("wq", [128, KT * DO], MMDT, isOutput=False)
    wk = nc.declare_dram_parameter("wk", [128, KT * DO], MMDT, isOutput=False)
    wv = nc.declare_dram_parameter("wv", [128, KT * DO], MMDT, isOutput=False)
    wo = nc.declare_dram_parameter("wo", [128, 2 * OUT], MMDT, isOutput=False)
    bq2 = nc.declare_dram_parameter("bq2", [128, 2], F32, isOutput=False)
    bv = nc.declare_dram_parameter("bv", [DO], F32, isOutput=False)
    bcol = nc.declare_dram_parameter("bcol", [128, TT], F32, isOutput=False)
    outT = nc.declare_dram_parameter("outT", [OUT, T], F16, isOutput=True)

    with tile.TileContext(nc) as tc:
        with (
            tc.tile_pool(name="cw", bufs=1) as cw,
            tc.tile_pool(name="stage", bufs=6) as stage,
            tc.tile_pool(name="persist", bufs=1) as persist,
            tc.tile_pool(name="small", bufs=2) as small,
            tc.tile_pool(name="ptp", bufs=(4 if dbg else 6)) as ptp,
            tc.tile_pool(name="px", bufs=1) as px,
            tc.tile_pool(name="ps_s", bufs=2, space="PSUM") as ps_s,
            tc.tile_pool(name="ps_mm", bufs=2, space="PSUM") as ps_mm,
        ):
            # ---- warm-up: junk matmuls to open the HAM clock gate while
            # the input DMAs are still in flight ----
            junk = cw.tile([128, 512], MMDT, tag="junk")
            nc.vector.memset(junk[:], 0.0)
            for i in range(N_WARM):
                wps = ps_s.tile([128, 1024], F32, tag="s", name=f"warm{i}")
                nc.tensor.matmul(
                    wps[:, :512], junk[:, 0:128], junk[:], start=True, stop=True
                )

            # ---- inputs (bf16, host pre-arranged to the SBUF layout) ----
            # the sync engine's preamble delays its first DMA descriptor to
            # ~7.5us; scalar/gpsimd come up ~3us earlier AND give parallel
            # descriptor streams + DMA rings, so the startup-critical loads
            # (wq, wk, first x half) are split across those two engines
            def load_w(tag, dram, shape, pat, eng=None):
                r = px.tile(shape, MMDT, tag=tag, name=f"r_{tag}")
                (eng or nc.sync).dma_start(out=r[:], in_=dram.rearrange(pat, kt=shape[1]))
                return r

            wq_r = load_w("wq", wq, [128, KT, DO], "p (kt m) -> p kt m", eng=nc.scalar)
            wk_r = load_w("wk", wk, [128, KT, DO], "p (kt m) -> p kt m", eng=nc.gpsimd)
            xr = []
            for kt in range(KT):
                r = px.tile([128, T], MMDT, tag=f"xr{kt}", name=f"r_xr{kt}")
                eng = nc.scalar if kt % 2 == 0 else nc.gpsimd
                eng.dma_start(
                    out=r[:, 0:1024], in_=xt[kt * 128:(kt + 1) * 128, 0:1024]
                )
                xr.append(r)
            wv_r = load_w("wv", wv, [128, KT, DO], "p (kt m) -> p kt m")

            # ---- constants ----
            bq_sb = cw.tile([128, 2], F32, tag="bq")
            nc.sync.dma_start(out=bq_sb[:], in_=bq2[:, :])
            bcol_sb = cw.tile([128, TT], F32, tag="bcol")
            nc.sync.dma_start(out=bcol_sb[:], in_=bcol[:, :])
            bv_sb = cw.tile([128, DO], F32, tag="bv")
            bv_ap = bv.ap()
            bv_bcast = bass.AP(tensor=bv_ap.tensor, offset=bv_ap.offset, ap=[[0, 128], [1, DO]])
            nc.sync.dma_start(out=bv_sb[:], in_=bv_bcast)
            ones_f = cw.tile([128, NH], F32, tag="ones")
            nc.vector.memset(ones_f[:], 1.0)

            # x second column-half, behind everything needed at stream start
            for kt in range(KT):
                nc.sync.dma_start(
                    out=xr[kt][:, 1024:2048],
                    in_=xt[kt * 128:(kt + 1) * 128, 1024:2048],
                )

            # ---- persistent activation tiles ----
            qt2 = [persist.tile([128, T], MMDT, tag=f"qt{mi}", name=f"qt{mi}") for mi in range(2)]
            kth = [persist.tile([128, T], MMDT, tag=f"kh{h}", name=f"kh{h}") for h in range(NH)]
            # per-head V' stride padded 65->128 so the PV LDWEIGHTS qualifies
            # for fast weight load (compiler FWL requires NumWeights==128)
            vp = persist.tile([128, TT, NH * 128], MMDT, tag="vp")
            nc.vector.memset(
                vp.rearrange("p tt (h c) -> p tt h c", c=128)[:, :, :, 65:128], 0.0
            )
            at2p = [persist.tile([128, T], MMDT, tag=f"atp{p}", name=f"atp{p}") for p in range(2)]
            for h in range(NH):
                lo, hi = ((64, 128) if h % 2 == 0 else (0, 64))
                nc.vector.memset(kth[h][lo:hi, :], 0.0)

            # ---- emission helpers ----
            def qk_group(w_r, mi, nb, dst, pool_tag="attn"):
                """One [128,512] projection psum group: 8 matmuls + biased copy."""
                pool = ps_s if pool_tag == "s" else ps_mm
                ps = pool.tile([128, 1024], F32, tag=pool_tag, name=f"ps_p{mi}_{nb}")
                for kt in range(KT):
                    nc.tensor.matmul(
                        ps[:, :512],
                        w_r[:, kt, mi * 128:(mi + 1) * 128],
                        xr[kt][:, nb * 512:(nb + 1) * 512],
                        start=(kt == 0),
                        stop=(kt == KT - 1),
                    )
                nc.vector.tensor_scalar_add(
                    dst[mi][:, nb * 512:(nb + 1) * 512], ps[:, :512], bq_sb[:, mi:mi + 1]
                )

            kt2 = [None, None]  # paired K^T staging (rows then split into kth)

            def build_kth(mi, c0=0, c1=T):
                # even head of the pair: rows 0:64 stay, odd head: rows 64:128
                h0, h1 = 2 * mi, 2 * mi + 1
                nc.vector.tensor_copy(out=kth[h0][0:64, c0:c1], in_=kt2[mi][0:64, c0:c1])
                nc.vector.tensor_copy(out=kth[h1][64:128, c0:c1], in_=kt2[mi][64:128, c0:c1])

            def v_group(tt):
                ps = ps_mm.tile([128, 1024], F32, tag="attn", name=f"ps_v{tt}")
                for kt in range(KT):
                    nc.tensor.matmul(
                        ps[:, :DO],
                        xr[kt][:, tt * 128:(tt + 1) * 128],
                        wv_r[:, kt, :],
                        start=(kt == 0),
                        stop=(kt == KT - 1),
                    )
                for h in range(NH):
                    nc.vector.tensor_tensor(
                        vp[:, tt, h * 128:h * 128 + 64],
                        ps[:, h * 64:(h + 1) * 64],
                        bv_sb[:, h * 64:(h + 1) * 64],
                        ADD,
                    )
                ones_ap = vp[:, tt, :].rearrange("p (h c) -> p h c", c=128)[:, :, 64:65]
                nc.gpsimd.tensor_copy(out=ones_ap, in_=ones_f[:, :, None])

            wo_r = []

            def load_wo():
                r = ptp.tile([128, 2, OUT], MMDT, tag="wo", name="r_wo")
                nc.sync.dma_start(out=r[:], in_=wo.rearrange("p (j n) -> p j n", j=2))
                wo_r.append(r)

            def emit_pv(h, attn_ps, kt, pt):
                # lhsT is the full 128-wide padded head block (FWL); psum
                # rows 65:128 receive zeros and are never read
                for half in range(2):
                    nc.tensor.matmul(
                        attn_ps[:, half * 512:(half + 1) * 512],
                        vp[:, kt, h * 128:(h + 1) * 128],
                        pt[:, half * 512:(half + 1) * 512],
                        start=(kt == 0),
                        stop=(kt == TT - 1),
                    )

            def normalize_a(qbp, h, attn_ps, direct=False):
                # stage psum out + build the 1/denom broadcast; the multiply
                # (phase b) is deferred ~2 units so the vector FIFO never
                # waits on the gpsimd broadcast mid-stream
                den = cw.tile([65, 1024], F32, tag="den", name=f"den{qbp}_{h}")
                nc.vector.tensor_copy(out=den[64:65, :], in_=attn_ps[64:65, :])
                acopy = None
                if not direct:
                    acopy = small.tile([64, 1024], F32, tag="acopy", name=f"ac{qbp}_{h}")
                    nc.vector.tensor_copy(out=acopy[:], in_=attn_ps[0:64, :])
                d0 = cw.tile([1, 1024], F32, tag="d0", name=f"d0{qbp}_{h}")
                nc.sync.dma_start(out=d0[:], in_=den[64:65, :])
                rec = cw.tile([1, 1024], F32, tag="rec", name=f"rec{qbp}_{h}")
                nc.vector.reciprocal_approx_fast(rec[:], d0[:])
                rb = small.tile([64, 1024], F32, tag="rb", name=f"rb{qbp}_{h}")
                nc.gpsimd.partition_broadcast(rb[:], rec[:])
                return (qbp, h, attn_ps if direct else None, acopy, rb)

            def normalize_b(pending):
                qbp, h, attn_ps, acopy, rb = pending
                src = attn_ps[0:64, :] if acopy is None else acopy[:]
                sl = slice(qbp * 1024, (qbp + 1) * 1024)
                if h % 2 == 0:
                    nc.vector.tensor_tensor(at2p[h // 2][0:64, sl], src, rb[:], MULT)
                else:
                    atmp = small.tile([64, 1024], MMDT, tag="atmp", name=f"atmp{qbp}_{h}")
                    nc.vector.tensor_tensor(atmp[:], src, rb[:], MULT)
                    nc.sync.dma_start(out=at2p[h // 2][64:128, sl], in_=atmp[:])

            def c_group(nt, tb, pool=None, copy_eng=None, dma_eng=None):
                pool = pool or ps_mm
                ps = pool.tile([128, 1024], F32, tag=("attn" if pool is ps_mm else "s"), name=f"ps_c{nt}_{tb}")
                for j in range(2):
                    nc.tensor.matmul(
                        ps[:, :512],
                        wo_r[0][:, j, nt * 128:(nt + 1) * 128],
                        at2p[j][:, tb * 512:(tb + 1) * 512],
                        start=(j == 0),
                        stop=(j == 1),
                    )
                o_sb = stage.tile([128, 512], F16, tag="stage", name="o_sb")
                if copy_eng is nc.scalar:
                    nc.scalar.copy(out=o_sb[:], in_=ps[:, :512])
                else:
                    nc.vector.tensor_copy(out=o_sb[:], in_=ps[:, :512])
                (dma_eng or nc.sync).dma_start(
                    out=outT[nt * 128:(nt + 1) * 128, tb * 512:(tb + 1) * 512],
                    in_=o_sb[:],
                )

            # ---- emission schedule ----
            kt2[0] = persist.tile([128, T], MMDT, tag="kt2a", name="kt2a")
            kt2[1] = persist.tile([128, T], MMDT, tag="kt2b", name="kt2b")
            # minimal pre-stream: Q0 for queries 0:1024, K0 keys 0:512
            qk_group(wq_r, 0, 0, qt2, pool_tag="s")
            qk_group(wq_r, 0, 1, qt2, pool_tag="s")
            qk_group(wk_r, 0, 0, kt2, pool_tag="s")
            build_kth(0, 0, 512)

            # fillers per unit index; each entry is a list of thunks
            fillers = {}
            fillers[0] = [lambda: qk_group(wk_r, 0, 1, kt2), lambda: v_group(0)]
            fillers[1] = [lambda: build_kth(0, 512, 1024), lambda: v_group(1)]
            fillers[2] = [lambda: qk_group(wk_r, 0, 2, kt2), lambda: v_group(2)]
            fillers[3] = [lambda: qk_group(wk_r, 0, 3, kt2), lambda: v_group(3)]
            fillers[4] = [lambda: build_kth(0, 1024, 2048), lambda: v_group(4)]
            for i in range(5, 16):
                fillers[i] = [lambda tt=i: v_group(tt)]
            # era2 (units 16-31): K1 + kth split first so the h1->h2
            # transition never waits on the vector queue, then Q1/Q0 tails
            era2_items = [
                lambda: qk_group(wk_r, 1, 0, kt2),
                lambda: qk_group(wk_r, 1, 1, kt2),
                lambda: qk_group(wk_r, 1, 2, kt2),
                lambda: qk_group(wk_r, 1, 3, kt2),
                lambda: build_kth(1),
                lambda: qk_group(wq_r, 1, 0, qt2),
                lambda: qk_group(wq_r, 1, 1, qt2),
                lambda: qk_group(wq_r, 0, 2, qt2),
                lambda: qk_group(wq_r, 0, 3, qt2),
            ]
            for i, item in enumerate(era2_items):
                fillers[17 + i] = [item]
            fillers[34] = [load_wo]
            # Q1's second query half rides in the exp-bound qbp1 phase
            fillers[66] = [lambda: qk_group(wq_r, 1, 2, qt2)]
            fillers[70] = [lambda: qk_group(wq_r, 1, 3, qt2)]

            c_work = [(nt, tb) for tb in range(2) for nt in range(OUT // 128)]

            # head order (0,1,3,2): the pair-1 heads finish with the EVEN
            # head, so the last normalize writes at2p rows 0:64 directly
            # (no cross-partition DMA hop on the tail critical path)
            units = [(qbp, h, kt) for qbp in range(2) for h in (0, 1, 3, 2) for kt in range(TT)]
            attn_tiles = {}
            prev = None
            pending_norm = None
            for idx, (qbp, h, kt) in enumerate(units):
                if kt == 0:
                    attn_tiles[(qbp, h)] = ps_mm.tile(
                        [128, 1024], F32, tag="attn", name=f"attn_{qbp}_{h}"
                    )
                s_ps = ps_s.tile([128, 1024], F32, tag="s", name=f"s_{qbp}_{h}_{kt}")
                for half in range(2):
                    nc.tensor.matmul(
                        s_ps[:, half * 512:(half + 1) * 512],
                        kth[h][:, kt * 128:(kt + 1) * 128],
                        qt2[h // 2][:, qbp * 1024 + half * 512:qbp * 1024 + (half + 1) * 512],
                        start=True,
                        stop=True,
                    )
                pt = ptp.tile([128, 1024], MMDT, tag="pt")
                nc.scalar.activation(
                    pt[:], s_ps[:], EXP, bias=bcol_sb[:, kt:kt + 1], scale=0.125
                )
                if prev is not None:
                    pq, ph, pk, ppt = prev
                    emit_pv(ph, attn_tiles[(pq, ph)], pk, ppt)
                    if pk == TT - 1:
                        pending_norm = normalize_a(pq, ph, attn_tiles.pop((pq, ph)))
                if pending_norm is not None and kt == 2:
                    normalize_b(pending_norm)
                    pending_norm = None
                for item in fillers.get(idx, []):
                    item()
                if idx >= 64 and 4 <= kt <= 14 and kt % 3 == 1 and c_work:
                    nt, tb = c_work.pop(0)
                    c_group(nt, tb)
                prev = (qbp, h, kt, pt)
            pq, ph, pk, ppt = prev
            emit_pv(ph, attn_tiles[(pq, ph)], pk, ppt)
            # final head (even): normalize straight from psum, split by
            # 512-column halves so tb=2 output projections start while the
            # second half still multiplies
            fq, fh, faps, _, frb = normalize_a(pq, ph, attn_tiles.pop((pq, ph)), direct=True)

            # tail: alternate psum pools and staging engines, spread DMAs
            tail_i = 0

            def tail_c(nt, tb):
                nonlocal tail_i
                pool = (ps_mm, ps_s)[tail_i % 2]
                copy_eng = (nc.scalar, nc.vector)[tail_i % 2]
                dma_eng = (nc.sync, nc.scalar, nc.gpsimd)[tail_i % 3]
                c_group(nt, tb, pool=pool, copy_eng=copy_eng, dma_eng=dma_eng)
                tail_i += 1

            while c_work:
                nt, tb = c_work.pop(0)
                tail_c(nt, tb)
            for tb in range(2, NB):
                half = slice((tb - 2) * 512, (tb - 1) * 512)
                nc.vector.tensor_tensor(
                    at2p[fh // 2][0:64, fq * 1024 + (tb - 2) * 512:fq * 1024 + (tb - 1) * 512],
                    faps[0:64, half],
                    frb[:, half],
                    MULT,
                )
                for nt in range(OUT // 128):
                    tail_c(nt, tb)

            if dbg:
                def dump32(dst, src_ap, shape, nm):
                    t = stage.tile(shape, F32, tag="dump", name=f"dump_{nm}")
                    nc.vector.tensor_copy(out=t[:], in_=src_ap)
                    nc.sync.dma_start(out=dst, in_=t[:])
                for h in range(NH):
                    dump32(dbg["d_qt"][h], qt2[h // 2][(h % 2) * 64:(h % 2) * 64 + 64, :], [64, T], f"qt{h}")
                    kt_rows = kth[h][0:64, :] if h % 2 == 0 else kth[h][64:128, :]
                    dump32(dbg["d_kt"][h], kt_rows, [64, T], f"kt{h}")
                    dump32(dbg["d_at"][h], at2p[h // 2][(h % 2) * 64:(h % 2) * 64 + 64, :], [64, T], f"at{h}")
                for tt in range(TT):
                    dump32(dbg["d_vp"][:, tt, :], vp[:, tt, :], [128, NH * 65], f"vp{tt}")


def _build():
    nc = bacc.Bacc(trn_type="TRN2")
    build_attention(nc)
    nc.compile()
    return nc


def _get_nc():
    if "nc" not in _CACHE:
        _CACHE["nc"] = _build()
    return _CACHE["nc"]


def make_in_maps(x, W_q, b_q, W_k, W_v, b_v, W_o, bias):
    import ml_dtypes
    bf16 = ml_dtypes.bfloat16
    in_maps = []
    xtb = [np.ascontiguousarray(x[b].T.astype(bf16)) for b in range(B)]
    wqb = W_q.astype(bf16)
    wkb = W_k.astype(bf16)
    wvb = W_v.astype(bf16)
    wob = W_o.astype(bf16)

    def warr(w, sl):
        # [D, DO] -> [128, KT*DO] matching the SBUF [p, kt, m] layout
        return np.ascontiguousarray(
            w[:, sl].reshape(KT, 128, DO).transpose(1, 0, 2).reshape(128, KT * DO)
        )

    for c in range(8):
        b, hg = divmod(c, 4)
        sl = slice(hg * DO, (hg + 1) * DO)
        # wo rows r = j*128 + two*64 + p -> [(two p)=128, j, n] flattened
        wo_arr = np.ascontiguousarray(
            wob[sl, :].reshape(2, 2, 64, OUT).transpose(1, 2, 0, 3).reshape(128, 2 * OUT)
        )
        in_maps.append({
            "xt": xtb[b],
            "wq": warr(wqb, sl),
            "wk": warr(wkb, sl),
            "wv": warr(wvb, sl),
            "wo": wo_arr,
            "bq2": np.ascontiguousarray(b_q[sl].reshape(2, 128).T),
            "bv": np.ascontiguousarray(b_v[sl]),
            "bcol": np.ascontiguousarray(bias.reshape(TT, 128).T),
        })
    return in_maps


def kernel(x, W_q, b_q, W_k, b_k, W_v, b_v, W_o, b_o, bias, **_ignored):
    x = np.asarray(x, dtype=np.float32)
    W_q = np.asarray(W_q, dtype=np.float32)
    W_k = np.asarray(W_k, dtype=np.float32)
    W_v = np.asarray(W_v, dtype=np.float32)
    W_o = np.asarray(W_o, dtype=np.float32)
    b_q = np.asarray(b_q, dtype=np.float32)
    b_v = np.asarray(b_v, dtype=np.float32)
    b_o = np.asarray(b_o, dtype=np.float32)
    bias = np.asarray(bias, dtype=np.float32)

    nc = _get_nc()
    in_maps = make_in_maps(x, W_q, b_q, W_k, W_v, b_v, W_o, bias)
    _CACHE["in_maps"] = in_maps
    res = run_bass_kernel_spmd(nc, in_maps, list(range(8)))
    out = np.zeros((B, T, OUT), dtype=np.float32)
    for c in range(8):
        out[c // 4] += np.asarray(res.results[c]["outT"], dtype=np.float32).T
    out += b_o
    return out


# revision 23
# speedup vs baseline: 1.0338x; 1.0338x over previous
"""Multi-head attention (B=2, T=2048, D=OUT=1024, H=16) on 8 TRN2 NeuronCores.

Sharding: data-parallel over batch (2 groups) x tensor-parallel over heads
(4 groups of 4 heads). Core c handles batch c//4, heads (c%4)*4..(c%4)*4+4.
Each core computes Q^T/K^T/V for its head group, streams softmax(QK^T)V
in transposed layout (keys on partitions), and a partial output projection
through its W_o row block. The host sums the 4 partials per batch and adds
b_o.

Device-side layout notes:
- x is fed transposed ([D, T]) so Q^T/K^T come straight out of the PE.
- The softmax denominator rides along as a 65th ones-column in V', so one
  matmul produces both attn_unnorm^T and the per-query denominator.
- Matmul operands are bf16 (fp32 PSUM accumulation): full PE clock and
  fast weight load; fp32r measured 2x slower.
- Every matmul contracts over K=128 (K=64 matmuls run at half clock).
  The K-side operand of each attention matmul is a per-head tile
  zero-padded to 128 rows.
- Emission is interleaved: one software-pipelined stream over 128
  attention units; filler work rides on non-transition units.

Perf notes (this revision):
- Junk warm-up matmuls (vector-memset source, no DMA dependency) keep the
  HAM clock gate open from ~4us so real matmuls run at 2.4 GHz.
- Host pre-arranges weights into the exact SBUF layout so all input DMAs
  are contiguous; x arrives in two column halves and the pre-stream is
  trimmed to Q0(cols 0:1024) + K0(keys 0:512) so the exp stream starts
  ~10us earlier.
- era2 builds K1/kth[2:4] before Q1 so the vector queue never blocks the
  h1->h2 transition; Q1's second query half moved into the qbp1 phase
  (which is exp-bound, not PE-bound).
- normalize runs broadcast+multiply on gpsimd (vector's FIFO previously
  serialized the PE's next-head dependencies behind it).
- Partial outputs are fp16 (halves the output traffic); host accumulates
  in fp32. Tail output projections alternate PSUM pools and staging
  engines and spread DMAs across queues.
"""

import numpy as np

import concourse.bass as bass
import concourse.mybir as mybir
import concourse.tile as tile
from concourse import bacc
from concourse.bass_utils import run_bass_kernel_spmd

B, T, D, OUT, H = 2, 2048, 1024, 1024, 16
DO = 256            # output columns per core (4 heads x 64)
DEPTH = 64
NH = 4              # heads per core
KT = D // 128       # 8 contraction tiles for the projections
TT = T // 128       # 16 key tiles
NB = T // 512       # 4 query/time blocks
F32 = mybir.dt.float32
F16 = mybir.dt.float16
BF16 = mybir.dt.bfloat16
MMDT = BF16
EXP = mybir.ActivationFunctionType.Exp
MULT = mybir.AluOpType.mult
ADD = mybir.AluOpType.add

N_WARM = 9          # junk matmuls to open the HAM clock gate

_CACHE = {}


def build_attention(nc, dbg=None):
    """Emit the attention program. dbg, when given, is a dict of extra DRAM
    debug outputs to dump intermediates into."""
    xt = nc.declare_dram_parameter("xt", [D, T], MMDT, isOutput=False)
    wq = nc.declare_dram_parameter("wq", [128, KT * DO], MMDT, isOutput=False)
    wk = nc.declare_dram_parameter("wk", [128, KT * DO], MMDT, isOutput=False)
    wv = nc.declare_dram_parameter("wv", [128, KT * DO], MMDT, isOutput=False)
    wo = nc.declare_dram_parameter("wo", [128, 2 * OUT], MMDT, isOutput=False)
    bq2 = nc.declare_dram_parameter("bq2", [128, 2], F32, isOutput=False)
    bv = nc.declare_dram_parameter("bv", [DO], F32, isOutput=False)
    bcol = nc.declare_dram_parameter("bcol", [128, TT], F32, isOutput=False)
    outT = nc.declare_dram_parameter("outT", [OUT, T], F16, isOutput=True)

    with tile.TileContext(nc) as tc:
        with (
            tc.tile_pool(name="cw", bufs=1) as cw,
            tc.tile_pool(name="stage", bufs=6) as stage,
            tc.tile_pool(name="persist", bufs=1) as persist,
            tc.tile_pool(name="small", bufs=2) as small,
            tc.tile_pool(name="ptp", bufs=(4 if dbg else 6)) as ptp,
            tc.tile_pool(name="px", bufs=1) as px,
            tc.tile_pool(name="ps_s", bufs=2, space="PSUM") as ps_s,
            tc.tile_pool(name="ps_mm", bufs=2, space="PSUM") as ps_mm,
        ):
            # ---- warm-up: junk matmuls to open the HAM clock gate while
            # the input DMAs are still in flight ----
            junk = cw.tile([128, 512], MMDT, tag="junk")
            nc.vector.memset(junk[:], 0.0)
            for i in range(N_WARM):
                wps = ps_s.tile([128, 1024], F32, tag="s", name=f"warm{i}")
                nc.tensor.matmul(
                    wps[:, :512], junk[:, 0:128], junk[:], start=True, stop=True
                )
            # dummy activation pulls the exp table load (~1.3us) off the
            # first real exp's critical path
            dummy_e = cw.tile([1, 8], F32, tag="dummy_e")
            nc.scalar.activation(dummy_e[:], junk[0:1, 0:8], EXP)

            # ---- inputs (bf16, host pre-arranged to the SBUF layout) ----
            # the sync engine's preamble delays its first DMA descriptor to
            # ~7.5us; scalar/gpsimd come up ~3us earlier AND give parallel
            # descriptor streams + DMA rings, so the startup-critical loads
            # (wq, wk, first x half) are split across those two engines
            def load_w(tag, dram, shape, pat, eng=None):
                r = px.tile(shape, MMDT, tag=tag, name=f"r_{tag}")
                (eng or nc.sync).dma_start(out=r[:], in_=dram.rearrange(pat, kt=shape[1]))
                return r

            wq_r = load_w("wq", wq, [128, KT, DO], "p (kt m) -> p kt m")
            wk_r = load_w("wk", wk, [128, KT, DO], "p (kt m) -> p kt m")
            xr = []
            for kt in range(KT):
                r = px.tile([128, T], MMDT, tag=f"xr{kt}", name=f"r_xr{kt}")
                nc.sync.dma_start(
                    out=r[:, 0:1024], in_=xt[kt * 128:(kt + 1) * 128, 0:1024]
                )
                xr.append(r)
            wv_r = load_w("wv", wv, [128, KT, DO], "p (kt m) -> p kt m")

            # ---- constants ----
            bq_sb = cw.tile([128, 2], F32, tag="bq")
            nc.sync.dma_start(out=bq_sb[:], in_=bq2[:, :])
            bcol_sb = cw.tile([128, TT], F32, tag="bcol")
            nc.sync.dma_start(out=bcol_sb[:], in_=bcol[:, :])
            bv_sb = cw.tile([128, DO], F32, tag="bv")
            bv_ap = bv.ap()
            bv_bcast = bass.AP(tensor=bv_ap.tensor, offset=bv_ap.offset, ap=[[0, 128], [1, DO]])
            nc.sync.dma_start(out=bv_sb[:], in_=bv_bcast)
            ones_f = cw.tile([128, NH], F32, tag="ones")
            nc.vector.memset(ones_f[:], 1.0)

            # x second column-half, behind everything needed at stream start
            for kt in range(KT):
                nc.sync.dma_start(
                    out=xr[kt][:, 1024:2048],
                    in_=xt[kt * 128:(kt + 1) * 128, 1024:2048],
                )

            # ---- persistent activation tiles ----
            qt2 = [persist.tile([128, T], MMDT, tag=f"qt{mi}", name=f"qt{mi}") for mi in range(2)]
            kth = [persist.tile([128, T], MMDT, tag=f"kh{h}", name=f"kh{h}") for h in range(NH)]
            # per-head V' stride padded 65->128 so the PV LDWEIGHTS qualifies
            # for fast weight load (compiler FWL requires NumWeights==128)
            vp = persist.tile([128, TT, NH * 128], MMDT, tag="vp")
            nc.vector.memset(
                vp.rearrange("p tt (h c) -> p tt h c", c=128)[:, :, :, 65:128], 0.0
            )
            at2p = [persist.tile([128, T], MMDT, tag=f"atp{p}", name=f"atp{p}") for p in range(2)]
            for h in range(NH):
                lo, hi = ((64, 128) if h % 2 == 0 else (0, 64))
                nc.vector.memset(kth[h][lo:hi, :], 0.0)

            # ---- emission helpers ----
            def qk_group(w_r, mi, nb, dst, pool_tag="attn"):
                """One [128,512] projection psum group: 8 matmuls + biased copy."""
                pool = ps_s if pool_tag == "s" else ps_mm
                ps = pool.tile([128, 1024], F32, tag=pool_tag, name=f"ps_p{mi}_{nb}")
                for kt in range(KT):
                    nc.tensor.matmul(
                        ps[:, :512],
                        w_r[:, kt, mi * 128:(mi + 1) * 128],
                        xr[kt][:, nb * 512:(nb + 1) * 512],
                        start=(kt == 0),
                        stop=(kt == KT - 1),
                    )
                nc.vector.tensor_scalar_add(
                    dst[mi][:, nb * 512:(nb + 1) * 512], ps[:, :512], bq_sb[:, mi:mi + 1]
                )

            kt2 = [None, None]  # paired K^T staging (rows then split into kth)

            def build_kth(mi, c0=0, c1=T):
                # even head of the pair: rows 0:64 stay, odd head: rows 64:128
                h0, h1 = 2 * mi, 2 * mi + 1
                nc.vector.tensor_copy(out=kth[h0][0:64, c0:c1], in_=kt2[mi][0:64, c0:c1])
                nc.vector.tensor_copy(out=kth[h1][64:128, c0:c1], in_=kt2[mi][64:128, c0:c1])

            def v_group(tt):
                ps = ps_mm.tile([128, 1024], F32, tag="attn", name=f"ps_v{tt}")
                for kt in range(KT):
                    nc.tensor.matmul(
                        ps[:, :DO],
                        xr[kt][:, tt * 128:(tt + 1) * 128],
                        wv_r[:, kt, :],
                        start=(kt == 0),
                        stop=(kt == KT - 1),
                    )
                for h in range(NH):
                    nc.vector.tensor_tensor(
                        vp[:, tt, h * 128:h * 128 + 64],
                        ps[:, h * 64:(h + 1) * 64],
                        bv_sb[:, h * 64:(h + 1) * 64],
                        ADD,
                    )
                ones_ap = vp[:, tt, :].rearrange("p (h c) -> p h c", c=128)[:, :, 64:65]
                nc.gpsimd.tensor_copy(out=ones_ap, in_=ones_f[:, :, None])

            wo_r = []

            def load_wo():
                r = ptp.tile([128, 2, OUT], MMDT, tag="wo", name="r_wo")
                nc.sync.dma_start(out=r[:], in_=wo.rearrange("p (j n) -> p j n", j=2))
                wo_r.append(r)

            def emit_pv(h, attn_ps, kt, pt):
                # lhsT is the full 128-wide padded head block (FWL); psum
                # rows 65:128 receive zeros and are never read
                for half in range(2):
                    nc.tensor.matmul(
                        attn_ps[:, half * 512:(half + 1) * 512],
                        vp[:, kt, h * 128:(h + 1) * 128],
                        pt[:, half * 512:(half + 1) * 512],
                        start=(kt == 0),
                        stop=(kt == TT - 1),
                    )

            def normalize_a(qbp, h, attn_ps, direct=False):
                # stage psum out + build the 1/denom broadcast; the multiply
                # (phase b) is deferred ~2 units so the vector FIFO never
                # waits on the gpsimd broadcast mid-stream
                den = cw.tile([65, 1024], F32, tag="den", name=f"den{qbp}_{h}")
                nc.vector.tensor_copy(out=den[64:65, :], in_=attn_ps[64:65, :])
                acopy = None
                if not direct:
                    acopy = small.tile([64, 1024], F32, tag="acopy", name=f"ac{qbp}_{h}")
                    nc.vector.tensor_copy(out=acopy[:], in_=attn_ps[0:64, :])
                d0 = cw.tile([1, 1024], F32, tag="d0", name=f"d0{qbp}_{h}")
                nc.sync.dma_start(out=d0[:], in_=den[64:65, :])
                rec = cw.tile([1, 1024], F32, tag="rec", name=f"rec{qbp}_{h}")
                nc.vector.reciprocal_approx_fast(rec[:], d0[:])
                rb = small.tile([64, 1024], F32, tag="rb", name=f"rb{qbp}_{h}")
                nc.gpsimd.partition_broadcast(rb[:], rec[:])
                return (qbp, h, attn_ps if direct else None, acopy, rb)

            def normalize_b(pending):
                qbp, h, attn_ps, acopy, rb = pending
                src = attn_ps[0:64, :] if acopy is None else acopy[:]
                sl = slice(qbp * 1024, (qbp + 1) * 1024)
                if h % 2 == 0:
                    nc.vector.tensor_tensor(at2p[h // 2][0:64, sl], src, rb[:], MULT)
                else:
                    atmp = small.tile([64, 1024], MMDT, tag="atmp", name=f"atmp{qbp}_{h}")
                    nc.vector.tensor_tensor(atmp[:], src, rb[:], MULT)
                    nc.sync.dma_start(out=at2p[h // 2][64:128, sl], in_=atmp[:])

            def c_group(nt, tb, pool=None, copy_eng=None, dma_eng=None):
                pool = pool or ps_mm
                ps = pool.tile([128, 1024], F32, tag=("attn" if pool is ps_mm else "s"), name=f"ps_c{nt}_{tb}")
                for j in range(2):
                    nc.tensor.matmul(
                        ps[:, :512],
                        wo_r[0][:, j, nt * 128:(nt + 1) * 128],
                        at2p[j][:, tb * 512:(tb + 1) * 512],
                        start=(j == 0),
                        stop=(j == 1),
                    )
                o_sb = stage.tile([128, 512], F16, tag="stage", name="o_sb")
                if copy_eng is nc.scalar:
                    nc.scalar.copy(out=o_sb[:], in_=ps[:, :512])
                else:
                    nc.vector.tensor_copy(out=o_sb[:], in_=ps[:, :512])
                (dma_eng or nc.sync).dma_start(
                    out=outT[nt * 128:(nt + 1) * 128, tb * 512:(tb + 1) * 512],
                    in_=o_sb[:],
                )

            # ---- emission schedule ----
            kt2[0] = persist.tile([128, T], MMDT, tag="kt2a", name="kt2a")
            kt2[1] = persist.tile([128, T], MMDT, tag="kt2b", name="kt2b")
            # minimal pre-stream: Q0 for queries 0:1024, K0 keys 0:512
            qk_group(wq_r, 0, 0, qt2, pool_tag="s")
            qk_group(wq_r, 0, 1, qt2, pool_tag="s")
            qk_group(wk_r, 0, 0, kt2, pool_tag="s")
            build_kth(0, 0, 512)

            # fillers per unit index; each entry is a list of thunks
            fillers = {}
            fillers[0] = [lambda: qk_group(wk_r, 0, 1, kt2), lambda: v_group(0)]
            fillers[1] = [lambda: build_kth(0, 512, 1024), lambda: v_group(1)]
            fillers[2] = [lambda: qk_group(wk_r, 0, 2, kt2), lambda: v_group(2)]
            fillers[3] = [lambda: qk_group(wk_r, 0, 3, kt2), lambda: v_group(3)]
            fillers[4] = [lambda: build_kth(0, 1024, 2048), lambda: v_group(4)]
            for i in range(5, 16):
                fillers[i] = [lambda tt=i: v_group(tt)]
            # era2 (units 16-31): K1 + kth split first so the h1->h2
            # transition never waits on the vector queue, then Q1/Q0 tails
            era2_items = [
                lambda: qk_group(wk_r, 1, 0, kt2),
                lambda: qk_group(wk_r, 1, 1, kt2),
                lambda: qk_group(wk_r, 1, 2, kt2),
                lambda: qk_group(wk_r, 1, 3, kt2),
                lambda: build_kth(1),
                lambda: qk_group(wq_r, 1, 0, qt2),
                lambda: qk_group(wq_r, 1, 1, qt2),
                lambda: qk_group(wq_r, 0, 2, qt2),
                lambda: qk_group(wq_r, 0, 3, qt2),
            ]
            for i, item in enumerate(era2_items):
                fillers[17 + i] = [item]
            fillers[34] = [load_wo]
            # Q1's second query half rides in the exp-bound qbp1 phase
            fillers[66] = [lambda: qk_group(wq_r, 1, 2, qt2)]
            fillers[70] = [lambda: qk_group(wq_r, 1, 3, qt2)]

            c_work = [(nt, tb) for tb in range(2) for nt in range(OUT // 128)]

            # head order (0,1,3,2): the pair-1 heads finish with the EVEN
            # head, so the last normalize writes at2p rows 0:64 directly
            # (no cross-partition DMA hop on the tail critical path)
            units = [(qbp, h, kt) for qbp in range(2) for h in (0, 1, 3, 2) for kt in range(TT)]
            attn_tiles = {}
            prev = None
            pending_norm = None
            for idx, (qbp, h, kt) in enumerate(units):
                if kt == 0:
                    attn_tiles[(qbp, h)] = ps_mm.tile(
                        [128, 1024], F32, tag="attn", name=f"attn_{qbp}_{h}"
                    )
                s_ps = ps_s.tile([128, 1024], F32, tag="s", name=f"s_{qbp}_{h}_{kt}")
                for half in range(2):
                    nc.tensor.matmul(
                        s_ps[:, half * 512:(half + 1) * 512],
                        kth[h][:, kt * 128:(kt + 1) * 128],
                        qt2[h // 2][:, qbp * 1024 + half * 512:qbp * 1024 + (half + 1) * 512],
                        start=True,
                        stop=True,
                    )
                pt = ptp.tile([128, 1024], MMDT, tag="pt")
                nc.scalar.activation(
                    pt[:], s_ps[:], EXP, bias=bcol_sb[:, kt:kt + 1], scale=0.125
                )
                if prev is not None:
                    pq, ph, pk, ppt = prev
                    emit_pv(ph, attn_tiles[(pq, ph)], pk, ppt)
                    if pk == TT - 1:
                        pending_norm = normalize_a(pq, ph, attn_tiles.pop((pq, ph)))
                if pending_norm is not None and kt == 2:
                    normalize_b(pending_norm)
                    pending_norm = None
                for item in fillers.get(idx, []):
                    item()
                if idx >= 64 and 4 <= kt <= 14 and kt % 3 == 1 and c_work:
                    nt, tb = c_work.pop(0)
                    c_group(nt, tb)
                prev = (qbp, h, kt, pt)
            pq, ph, pk, ppt = prev
            emit_pv(ph, attn_tiles[(pq, ph)], pk, ppt)
            # final head (even): normalize straight from psum, split by
            # 512-column halves so tb=2 output projections start while the
            # second half still multiplies
            fq, fh, faps, _, frb = normalize_a(pq, ph, attn_tiles.pop((pq, ph)), direct=True)

            # tail: alternate psum pools and staging engines, spread DMAs
            tail_i = 0

            def tail_c(nt, tb):
                nonlocal tail_i
                pool = (ps_mm, ps_s)[tail_i % 2]
                copy_eng = (nc.scalar, nc.vector)[tail_i % 2]
                dma_eng = (nc.sync, nc.scalar, nc.gpsimd)[tail_i % 3]
                c_group(nt, tb, pool=pool, copy_eng=copy_eng, dma_eng=dma_eng)
                tail_i += 1

            while c_work:
                nt, tb = c_work.pop(0)
                tail_c(nt, tb)
            for tb in range(2, NB):
                half = slice((tb - 2) * 512, (tb - 1) * 512)
                nc.vector.tensor_tensor(
                    at2p[fh // 2][0:64, fq * 1024 + (tb - 2) * 512:fq * 1024 + (tb - 1) * 512],
                    faps[0:64, half],
                    frb[:, half],
                    MULT,
                )
                for nt in range(OUT // 128):
                    tail_c(nt, tb)

            if dbg:
                def dump32(dst, src_ap, shape, nm):
                    t = stage.tile(shape, F32, tag="dump", name=f"dump_{nm}")
                    nc.vector.tensor_copy(out=t[:], in_=src_ap)
                    nc.sync.dma_start(out=dst, in_=t[:])
                for h in range(NH):
                    dump32(dbg["d_qt"][h], qt2[h // 2][(h % 2) * 64:(h % 2) * 64 + 64, :], [64, T], f"qt{h}")
                    kt_rows = kth[h][0:64, :] if h % 2 == 0 else kth[h][64:128, :]
                    dump32(dbg["d_kt"][h], kt_rows, [64, T], f"kt{h}")
                    dump32(dbg["d_at"][h], at2p[h // 2][(h % 2) * 64:(h % 2) * 64 + 64, :], [64, T], f"at{h}")
                for tt in range(TT):
                    dump32(dbg["d_vp"][:, tt, :], vp[:, tt, :], [128, NH * 65], f"vp{tt}")


def _build():
    nc = bacc.Bacc(trn_type="TRN2")
    build_attention(nc)
    nc.compile()
    return nc


def _get_nc():
    if "nc" not in _CACHE:
        _CACHE["nc"] = _build()
    return _CACHE["nc"]


def make_in_maps(x, W_q, b_q, W_k, W_v, b_v, W_o, bias):
    import ml_dtypes
    bf16 = ml_dtypes.bfloat16
    in_maps = []
    xtb = [np.ascontiguousarray(x[b].T.astype(bf16)) for b in range(B)]
    wqb = W_q.astype(bf16)
    wkb = W_k.astype(bf16)
    wvb = W_v.astype(bf16)
    wob = W_o.astype(bf16)

    def warr(w, sl):
        # [D, DO] -> [128, KT*DO] matching the SBUF [p, kt, m] layout
        return np.ascontiguousarray(
            w[:, sl].reshape(KT, 128, DO).transpose(1, 0, 2).reshape(128, KT * DO)
        )

    for c in range(8):
        b, hg = divmod(c, 4)
        sl = slice(hg * DO, (hg + 1) * DO)
        # wo rows r = j*128 + two*64 + p -> [(two p)=128, j, n] flattened
        wo_arr = np.ascontiguousarray(
            wob[sl, :].reshape(2, 2, 64, OUT).transpose(1, 2, 0, 3).reshape(128, 2 * OUT)
        )
        in_maps.append({
            "xt": xtb[b],
            "wq": warr(wqb, sl),
            "wk": warr(wkb, sl),
            "wv": warr(wvb, sl),
            "wo": wo_arr,
            "bq2": np.ascontiguousarray(b_q[sl].reshape(2, 128).T),
            "bv": np.ascontiguousarray(b_v[sl]),
            "bcol": np.ascontiguousarray(bias.reshape(TT, 128).T),
        })
    return in_maps


def kernel(x, W_q, b_q, W_k, b_k, W_v, b_v, W_o, b_o, bias, **_ignored):
    x = np.asarray(x, dtype=np.float32)
    W_q = np.asarray(W_q, dtype=np.float32)
    W_k = np.asarray(W_k, dtype=np.float32)
    W_v = np.asarray(W_v, dtype=np.float32)
    W_o = np.asarray(W_o, dtype=np.float32)
    b_q = np.asarray(b_q, dtype=np.float32)
    b_v = np.asarray(b_v, dtype=np.float32)
    b_o = np.asarray(b_o, dtype=np.float32)
    bias = np.asarray(bias, dtype=np.float32)

    nc = _get_nc()
    in_maps = make_in_maps(x, W_q, b_q, W_k, W_v, b_v, W_o, bias)
    _CACHE["in_maps"] = in_maps
    res = run_bass_kernel_spmd(nc, in_maps, list(range(8)))
    out = np.zeros((B, T, OUT), dtype=np.float32)
    for c in range(8):
        out[c // 4] += np.asarray(res.results[c]["outT"], dtype=np.float32).T
    out += b_o
    return out
